# revision 41
# baseline (speedup 1.0000x reference)
"""Trainium2 Bass kernel for BatchAllTripletWithClustersLossSemiHard (v9).

Strategy (data-parallel over same-label pairs, 8 cores):
  Only (i,j) pairs with equal labels contribute.  The global pair list is
  built class-contiguously and split into 8 equal consecutive slices, so
  each core holds pairs from <=4 label classes.  Per core the k-axis is
  permuted so its own classes' columns come first: every excluded k
  (semi-hard rank parity, k==i, k==j) then lands in a fixed [0, WCAP)
  window, uniform across cores (SPMD program is identical; tables differ).

  Per pair-row p over permuted k:
      z[p,k] = V[i_p,k] + cvec_p + madd[p,k]
  with V[a,k] = w_a*(2 x_a.x_k - (|x_k|^2 - 512)) computed on device from
  fp8e4m3 embeddings (X and 2*w*x_a; the 512 centering + host-exact bias
  keep total loss error ~5e-4); the norm row rides as a separate C=1 bf16
  matmul.  cvec_p = w*(1 - 512 - V_ij) is computed exactly on host (f32).
  madd in {0, -4096} covers only the [0, WCAP) own-class window via an
  ident x madd matmul (bf16 in SBUF; fp8 on the wire via SWDGE cast).

  Inputs ride 3 DMA queues as whole contiguous [128, N] tensors (column-
  sliced or <128-partition transfers hit a ~25-50 GB/s slow path); PE
  warmup matmuls bridge the ~3.5us DMA latency so the HAM clock is warm
  for the chunk matmuls.  Per chunk of 128 pairs: one fp8/bf16 one-hot
  matmul broadcasts V rows, one narrow matmul adds the mask window;
  relu+bias+row-sum alternates between ScalarE (activation, accum_out)
  and DVE (scalar_tensor_tensor add/max, accum_out); counts alternate
  between DVE is_gt (accum) and ScalarE Sign(z-EPS) (accum encodes
  2*count-B, decoded on host).  Raw [128, 2*NCH] partials are DMA'd out;
  the host does the final scalar reduction.
"""

import numpy as np
import ml_dtypes

import concourse.bass as bass
import concourse.tile as tile
from concourse import bacc, mybir
from concourse.bass_utils import run_bass_kernel_spmd

EPS = 1e-8
NEG = -4096.0
COFF = 512.0  # |x|^2 centering offset
NCORES = 8
P = 128
NDC_V = 5  # contraction chunks for the V matmul (D+1 rows split in 5)
# per-chunk (relu engine, count variant): balance ScalarE vs DVE load
CHUNK_PLAN = [
    ("SC", "TS"), ("SC", "TS"), ("DVE", "SGN"),
    ("SC", "TS"), ("DVE", "SGN"), ("SC", "TS"),
    ("DVE", "TS"), ("SC", "TS"), ("DVE", "SGN"),
]
F32 = mybir.dt.float32
BF = mybir.dt.bfloat16
F8 = mybir.dt.float8e5
F8E4 = mybir.dt.float8e4


def _host_prep(embeddings, labels, clusters, weights):
    x = np.ascontiguousarray(np.asarray(embeddings, dtype=np.float32))
    labels = np.asarray(labels).astype(np.int64)
    clusters = np.asarray(clusters).astype(np.int64)
    weights = np.asarray(weights).astype(np.float64)
    B, D = x.shape

    leq = labels[None, :] == labels[:, None]
    rank = np.cumsum(leq.astype(np.int64), axis=1) - 1
    first = leq & (rank % 2 == 1)
    second = leq & (rank % 2 == 0)
    pbase = ~first
    qbase = ~second

    xd = x.astype(np.float64)
    sq = np.einsum("bd,bd->b", xd, xd)
    wper = weights[labels]

    # class-contiguous global pair list
    classes = [np.where(labels == g)[0] for g in range(int(labels.max()) + 1)]
    classes = [m for m in classes if len(m) > 0]
    all_pairs = []
    for gi, m in enumerate(classes):
        for i in m:
            for j in m:
                if j != i:
                    all_pairs.append((gi, int(i), int(j)))
    total = len(all_pairs)
    Q = (total + NCORES - 1) // NCORES
    NP = ((Q + P - 1) // P) * P
    NCH = NP // P

    cores = []
    for c in range(NCORES):
        pairs = all_pairs[c * Q:min((c + 1) * Q, total)]
        own_cls = sorted({g for g, i, j in pairs})
        own_cols = [int(k) for g in own_cls for k in classes[g]]
        anchors = sorted({i for g, i, j in pairs})
        cores.append(dict(pairs=pairs, own_cols=own_cols, anchors=anchors))
    MA = max(len(cc["anchors"]) for cc in cores)
    WCAP = ((max(len(cc["own_cols"]) for cc in cores) + 15) // 16) * 16
    assert MA <= P and WCAP <= B

    tables = []
    for c in range(NCORES):
        cc = cores[c]
        pairs, own_cols, anchors = cc["pairs"], cc["own_cols"], cc["anchors"]
        own_set = set(own_cols)
        perm = own_cols + [k for k in range(B) if k not in own_set]
        colpos = np.empty(B, np.int64)
        colpos[np.array(perm)] = np.arange(B)
        aidx = {i: a for a, i in enumerate(anchors)}

        sel = np.zeros((P, NP), ml_dtypes.float8_e5m2)
        # cols [0:NCH) = cvec;  [NCH:2NCH) = cvec - EPS (Sign-count bias)
        cvec = np.full((P, 2 * NCH), NEG, np.float32)
        madd = np.zeros((P, NCH * WCAP), ml_dtypes.float8_e5m2)
        for r, (g, i, j) in enumerate(pairs):
            ch, row = divmod(r, P)
            sel[aidx[i], ch * P + row] = 1.0
            vij = 2.0 * float(xd[i] @ xd[j]) - float(sq[j])
            cv = wper[i] * (1.0 - COFF - vij)
            cvec[row, ch] = np.float32(cv)
            cvec[row, NCH + ch] = np.float32(cv - EPS)
            base = pbase[i] if clusters[i] == clusters[j] else qbase[i]
            mask = base.copy()
            mask[i] = False
            mask[j] = False
            for k in classes[g]:
                if not mask[k]:
                    madd[row, ch * WCAP + colpos[k]] = NEG

        # xmy: [D, MA] = 2*w_a*x_a, D-chunked [128, NDC*MA]
        a_arr = np.array(anchors, np.int64)
        prm = np.array(perm)
        NDC = D // P
        xmy = np.zeros((D, MA), np.float64)
        xmy[:, :len(anchors)] = 2.0 * wper[a_arr][None, :] * xd[a_arr].T
        xmy = xmy.astype(ml_dtypes.float8_e4m3)
        megaM = np.concatenate(
            [xmy[dc * P:(dc + 1) * P, :] for dc in range(NDC)], axis=1)
        xt = x.T[:, prm].astype(ml_dtypes.float8_e4m3)
        xchunks = {
            "megaXa": np.ascontiguousarray(np.concatenate(
                [xt[dc * P:(dc + 1) * P, :] for dc in range(NDC // 2)],
                axis=1)),
            "megaXb": np.ascontiguousarray(np.concatenate(
                [xt[dc * P:(dc + 1) * P, :] for dc in range(NDC // 2, NDC)],
                axis=1)),
        }
        nn = np.zeros((1, B + MA), ml_dtypes.bfloat16)
        nn[0, :B] = (sq[prm] - COFF).astype(np.float32) \
            .astype(ml_dtypes.bfloat16)
        nn[0, B:B + len(anchors)] = (-wper[a_arr]).astype(ml_dtypes.bfloat16)
        tables.append(dict(
            **xchunks,
            megaM=np.ascontiguousarray(megaM),
            selT=np.ascontiguousarray(sel),
            maddT=np.ascontiguousarray(madd),
            cvecT=np.ascontiguousarray(cvec),
            nnT=np.ascontiguousarray(nn),
            identT=np.eye(P, dtype=ml_dtypes.float8_e5m2),
        ))
    return tables, NP, MA, WCAP, B, D // P


def _build_program(NCH, MA, WCAP, B, NDC):
    NWARM = 15
    nc = bacc.Bacc("TRN2", target_bir_lowering=False, debug=False,
                   num_devices=NCORES)

    megaXa = nc.dram_tensor("megaXa", [P, (NDC // 2) * B], F8E4,
                            kind="ExternalInput")
    megaXb = nc.dram_tensor("megaXb", [P, (NDC - NDC // 2) * B], F8E4,
                            kind="ExternalInput")
    megaM = nc.dram_tensor("megaM", [P, NDC * MA], F8E4, kind="ExternalInput")
    selT = nc.dram_tensor("selT", [P, NCH * P], F8, kind="ExternalInput")
    maddT = nc.dram_tensor("maddT", [P, NCH * WCAP], F8, kind="ExternalInput")
    cvecT = nc.dram_tensor("cvecT", [P, 2 * NCH], F32, kind="ExternalInput")
    nnT = nc.dram_tensor("nnT", [1, B + MA], BF, kind="ExternalInput")
    identT = nc.dram_tensor("identT", [P, P], F8, kind="ExternalInput")
    out_s = nc.dram_tensor("out_s", [P, 2 * NCH], F32, kind="ExternalOutput")

    with tile.TileContext(nc) as tc:
        with (
            tc.tile_pool(name="cst", bufs=1) as cst,
            tc.tile_pool(name="sm", bufs=4) as sm,
            tc.tile_pool(name="wps", bufs=1, space="PSUM") as wps,
            tc.tile_pool(name="vps", bufs=1, space="PSUM") as vps,
            tc.tile_pool(name="gps", bufs=6, space="PSUM") as gps,
        ):
            # scratch for PE warmup + ACT table preload (no input deps)
            scratch = cst.tile([P, 256], BF)
            nc.vector.memset(scratch[:], 0.25)
            zeros = cst.tile([P, B], BF)
            nc.vector.memset(zeros[:], 0.0)
            zl = cst.tile([P, NCH * B], BF)
            sacc = cst.tile([P, 2 * NCH], F32)

            # input DMAs: contiguous full-tensor DMAs, X split on 2 queues
            mXa = cst.tile([P, (NDC // 2) * B], F8E4)
            nc.sync.dma_start(mXa[:], megaXa[:, :])
            mM = cst.tile([P, NDC * MA], F8E4)
            nc.scalar.dma_start(mM[:], megaM[:, :])
            mXb = cst.tile([P, (NDC - NDC // 2) * B], F8E4)
            nc.scalar.dma_start(mXb[:], megaXb[:, :])
            # tables ride the SWDGE queue (fp8 on the wire, bf16 in SBUF);
            # emission latency naturally defers them behind the X transfers
            nn_sb = cst.tile([1, B + MA], BF)
            nc.gpsimd.dma_start(nn_sb[:], nnT[:, :])
            madd_sb = cst.tile([P, NCH * WCAP], BF)
            nc.gpsimd.dma_start(madd_sb[:], maddT[:, :])
            ident_sb = cst.tile([P, P], BF)
            nc.gpsimd.dma_start(ident_sb[:], identT[:, :])
            sel_sb = cst.tile([P, NCH * P], BF)
            nc.gpsimd.dma_start(sel_sb[:], selT[:, :])
            cvec_sb = cst.tile([P, 2 * NCH], F32)
            nc.gpsimd.dma_start(cvec_sb[:], cvecT[:, :])

            # trigger ACT table load early + warm the PE clock
            tinya = sm.tile([P, 8], BF, tag="tinya")
            nc.scalar.activation(tinya[:], scratch[:, 0:8],
                                 mybir.ActivationFunctionType.Relu)
            warm = wps.tile([P, 256], F32)
            for _ in range(NWARM):
                nc.tensor.matmul(warm[:], lhsT=scratch[:, 0:P],
                                 rhs=scratch[:], start=True, stop=True)

            # V[a,k] = w_a*(2 x_a.x_k - (|x_k|^2 - 512)) in PSUM
            v_sb = cst.tile([P, B], BF)
            nc.gpsimd.memset(v_sb[:], 0.0)
            v_psum = vps.tile([MA, B], F32)
            h2 = NDC // 2
            for dc in range(NDC):
                rhs = (mXa[:, dc * B:(dc + 1) * B] if dc < h2
                       else mXb[:, (dc - h2) * B:(dc - h2 + 1) * B])
                nc.tensor.matmul(v_psum[:],
                                 lhsT=mM[:, dc * MA:(dc + 1) * MA],
                                 rhs=rhs, start=(dc == 0), stop=False)
            nc.tensor.matmul(v_psum[:], lhsT=nn_sb[0:1, B:B + MA],
                             rhs=nn_sb[0:1, 0:B], start=False, stop=True)
            nc.vector.tensor_copy(v_sb[0:MA, :], v_psum[:, :])

            # per-chunk pipeline; sacc col 2c = row-sum, 2c+1 = count
            for c in range(NCH):
                vg = gps.tile([P, B], F32, tag="vg")
                nc.tensor.matmul(vg[:], lhsT=sel_sb[:, c * P:(c + 1) * P],
                                 rhs=v_sb[:], start=True, stop=False)
                nc.tensor.matmul(vg[:, 0:WCAP], lhsT=ident_sb[:],
                                 rhs=madd_sb[:, c * WCAP:(c + 1) * WCAP],
                                 start=False, stop=True)
                zc = zl[:, c * B:(c + 1) * B]
                relu_v, cnt_v = CHUNK_PLAN[c % len(CHUNK_PLAN)]
                if relu_v == "SC":
                    nc.scalar.activation(zc, vg[:],
                                         mybir.ActivationFunctionType.Relu,
                                         bias=cvec_sb[:, c:c + 1], scale=1.0,
                                         accum_out=sacc[:, 2 * c:2 * c + 1])
                else:
                    nc.vector.scalar_tensor_tensor(
                        zc, in0=vg[:], scalar=cvec_sb[:, c:c + 1],
                        in1=zeros[:], op0=mybir.AluOpType.add,
                        op1=mybir.AluOpType.max,
                        accum_out=sacc[:, 2 * c:2 * c + 1])
                cd = sm.tile([P, B], BF, tag="cd")
                if cnt_v == "TS":
                    nc.vector.tensor_scalar(cd[:], zc, float(EPS), None,
                                            op0=mybir.AluOpType.is_gt,
                                            op1=mybir.AluOpType.add,
                                            accum_out=sacc[:, 2 * c + 1:2 * c + 2])
                else:  # SGN: sign(z - EPS) accum = 2*count - B
                    nc.scalar.activation(cd[:], vg[:],
                                         mybir.ActivationFunctionType.Sign,
                                         bias=cvec_sb[:, NCH + c:NCH + c + 1],
                                         scale=1.0,
                                         accum_out=sacc[:, 2 * c + 1:2 * c + 2])
                if c == NCH - 2:
                    nc.sync.dma_start(out_s[:, 0:2 * (NCH - 1)],
                                      sacc[:, 0:2 * (NCH - 1)])
            nc.sync.dma_start(out_s[:, 2 * (NCH - 1):],
                              sacc[:, 2 * (NCH - 1):])

    nc.compile()
    return nc


def run(embeddings, labels, clusters, weights, trace=False):
    tables, NP, MA, WCAP, B, NDC = _host_prep(embeddings, labels, clusters,
                                              weights)
    NCH = NP // P
    nc = _build_program(NCH, MA, WCAP, B, NDC)
    res = run_bass_kernel_spmd(nc, tables, core_ids=list(range(NCORES)),
                               trace=trace)
    S = 0.0
    C = 0.0
    for r in res.results:
        o = np.asarray(r["out_s"], np.float64)
        S += float(o[:, 0::2].sum())
        for c in range(NCH):
            col = float(o[:, 2 * c + 1].sum())
            if CHUNK_PLAN[c % len(CHUNK_PLAN)][1] == "SGN":
                C += (col + P * B) / 2.0
            else:
                C += col
    loss = np.float32(np.float32(S) / np.float32(C + EPS))
    return np.asarray(loss, dtype=np.float32), res


def kernel(embeddings, labels, clusters, weights):
    loss, _ = run(embeddings, labels, clusters, weights)
    return loss


# revision 43
# speedup vs baseline: 1.1908x; 1.1908x over previous
"""Trainium2 Bass kernel for BatchAllTripletWithClustersLossSemiHard (v9).

Strategy (data-parallel over same-label pairs, 8 cores):
  Only (i,j) pairs with equal labels contribute.  The global pair list is
  built class-contiguously and split into 8 equal consecutive slices, so
  each core holds pairs from <=4 label classes.  Per core the k-axis is
  permuted so its own classes' columns come first: every excluded k
  (semi-hard rank parity, k==i, k==j) then lands in a fixed [0, WCAP)
  window, uniform across cores (SPMD program is identical; tables differ).

  Per pair-row p over permuted k:
      z[p,k] = V[i_p,k] + cvec_p + madd[p,k]
  with V[a,k] = w_a*(2 x_a.x_k - (|x_k|^2 - 512)) computed on device from
  fp8e4m3 embeddings (X and 2*w*x_a; the 512 centering + host-exact bias
  keep total loss error ~5e-4); the norm row rides as a separate C=1 bf16
  matmul.  cvec_p = w*(1 - 512 - V_ij) is computed exactly on host (f32).
  madd in {0, -4096} covers only the [0, WCAP) own-class window via an
  ident x madd matmul (bf16 in SBUF; fp8 on the wire via SWDGE cast).

  Inputs ride 3 DMA queues as whole contiguous [128, N] tensors (column-
  sliced or <128-partition transfers hit a ~25-50 GB/s slow path); PE
  warmup matmuls bridge the ~3.5us DMA latency so the HAM clock is warm
  for the chunk matmuls.  Per chunk of 128 pairs: one fp8/bf16 one-hot
  matmul broadcasts V rows, one narrow matmul adds the mask window;
  relu+bias+row-sum alternates between ScalarE (activation, accum_out)
  and DVE (scalar_tensor_tensor add/max, accum_out); counts alternate
  between DVE is_gt (accum) and ScalarE Sign(z-EPS) (accum encodes
  2*count-B, decoded on host).  Raw [128, 2*NCH] partials are DMA'd out;
  the host does the final scalar reduction.
"""

import numpy as np
import ml_dtypes

import concourse.bass as bass
import concourse.tile as tile
from concourse import bacc, mybir
from concourse.bass_utils import run_bass_kernel_spmd

EPS = 1e-8
NEG = -4096.0
COFF = 512.0  # |x|^2 centering offset
NCORES = 8
P = 128
NDC_V = 5  # contraction chunks for the V matmul (D+1 rows split in 5)
# per-chunk (relu engine, count variant): balance ScalarE vs DVE load
CHUNK_PLAN = [
    ("SC", "TS"), ("SC", "TS"), ("DVE", "SGN"),
    ("SC", "TS"), ("DVE", "SGN"), ("SC", "TS"),
    ("DVE", "TS"), ("SC", "TS"), ("DVE", "SGN"),
]
F32 = mybir.dt.float32
BF = mybir.dt.bfloat16
F8 = mybir.dt.float8e5
F8E4 = mybir.dt.float8e4


def _host_prep(embeddings, labels, clusters, weights):
    x = np.ascontiguousarray(np.asarray(embeddings, dtype=np.float32))
    labels = np.asarray(labels).astype(np.int64)
    clusters = np.asarray(clusters).astype(np.int64)
    weights = np.asarray(weights).astype(np.float64)
    B, D = x.shape

    leq = labels[None, :] == labels[:, None]
    rank = np.cumsum(leq.astype(np.int64), axis=1) - 1
    first = leq & (rank % 2 == 1)
    second = leq & (rank % 2 == 0)
    pbase = ~first
    qbase = ~second

    xd = x.astype(np.float64)
    sq = np.einsum("bd,bd->b", xd, xd)
    wper = weights[labels]

    # class-contiguous global pair list
    classes = [np.where(labels == g)[0] for g in range(int(labels.max()) + 1)]
    classes = [m for m in classes if len(m) > 0]
    all_pairs = []
    for gi, m in enumerate(classes):
        for i in m:
            for j in m:
                if j != i:
                    all_pairs.append((gi, int(i), int(j)))
    total = len(all_pairs)
    Q = (total + NCORES - 1) // NCORES
    NP = ((Q + P - 1) // P) * P
    NCH = NP // P

    cores = []
    for c in range(NCORES):
        pairs = all_pairs[c * Q:min((c + 1) * Q, total)]
        own_cls = sorted({g for g, i, j in pairs})
        own_cols = [int(k) for g in own_cls for k in classes[g]]
        anchors = sorted({i for g, i, j in pairs})
        cores.append(dict(pairs=pairs, own_cols=own_cols, anchors=anchors))
    MA = max(len(cc["anchors"]) for cc in cores)
    WCAP = ((max(len(cc["own_cols"]) for cc in cores) + 15) // 16) * 16
    assert MA <= P and WCAP <= B

    tables = []
    for c in range(NCORES):
        cc = cores[c]
        pairs, own_cols, anchors = cc["pairs"], cc["own_cols"], cc["anchors"]
        own_set = set(own_cols)
        perm = own_cols + [k for k in range(B) if k not in own_set]
        colpos = np.empty(B, np.int64)
        colpos[np.array(perm)] = np.arange(B)
        aidx = {i: a for a, i in enumerate(anchors)}

        sel = np.zeros((P, NP), ml_dtypes.float8_e5m2)
        # cols [0:NCH)=cvec; [NCH:2NCH)=cvec-EPS (Sign bias);
        # [2NCH:3NCH)=-cvec (shifted-relu max operand);
        # [3NCH:4NCH)=EPS-cvec (count threshold on shifted zl)
        cvec = np.full((P, 4 * NCH), NEG, np.float32)
        cvec[:, 2 * NCH:3 * NCH] = -NEG
        cvec[:, 3 * NCH:4 * NCH] = np.float32(EPS - NEG)
        madd = np.zeros((P, NCH * WCAP), ml_dtypes.float8_e5m2)
        for r, (g, i, j) in enumerate(pairs):
            ch, row = divmod(r, P)
            sel[aidx[i], ch * P + row] = 1.0
            vij = 2.0 * float(xd[i] @ xd[j]) - float(sq[j])
            cv = wper[i] * (1.0 - COFF - vij)
            cvec[row, ch] = np.float32(cv)
            cvec[row, NCH + ch] = np.float32(cv - EPS)
            cvec[row, 2 * NCH + ch] = np.float32(-cv)
            cvec[row, 3 * NCH + ch] = np.float32(EPS - cv)
            base = pbase[i] if clusters[i] == clusters[j] else qbase[i]
            mask = base.copy()
            mask[i] = False
            mask[j] = False
            for k in classes[g]:
                if not mask[k]:
                    madd[row, ch * WCAP + colpos[k]] = NEG

        # xmy: [D, MA] = 2*w_a*x_a, D-chunked [128, NDC*MA]
        a_arr = np.array(anchors, np.int64)
        prm = np.array(perm)
        NDC = D // P
        xmy = np.zeros((D, MA), np.float64)
        xmy[:, :len(anchors)] = 2.0 * wper[a_arr][None, :] * xd[a_arr].T
        xmy = xmy.astype(ml_dtypes.float8_e4m3)
        megaM = np.concatenate(
            [xmy[dc * P:(dc + 1) * P, :] for dc in range(NDC)], axis=1)
        xt = x.T[:, prm].astype(ml_dtypes.float8_e4m3)
        xchunks = {
            "megaXa": np.ascontiguousarray(np.concatenate(
                [xt[dc * P:(dc + 1) * P, :] for dc in range(NDC // 2)],
                axis=1)),
            "megaXb": np.ascontiguousarray(np.concatenate(
                [xt[dc * P:(dc + 1) * P, :] for dc in range(NDC // 2, NDC)],
                axis=1)),
        }
        nn = np.zeros((1, B + MA), ml_dtypes.bfloat16)
        nn[0, :B] = (sq[prm] - COFF).astype(np.float32) \
            .astype(ml_dtypes.bfloat16)
        nn[0, B:B + len(anchors)] = (-wper[a_arr]).astype(ml_dtypes.bfloat16)
        tables.append(dict(
            **xchunks,
            megaM=np.ascontiguousarray(megaM),
            selT=np.ascontiguousarray(sel),
            maddT=np.ascontiguousarray(madd),
            cvecT=np.ascontiguousarray(cvec),
            nnT=np.ascontiguousarray(nn),
            identT=np.eye(P, dtype=ml_dtypes.float8_e5m2),
        ))
    return tables, NP, MA, WCAP, B, D // P


def _build_program(NCH, MA, WCAP, B, NDC):
    NWARM = 15
    nc = bacc.Bacc("TRN2", target_bir_lowering=False, debug=False,
                   num_devices=NCORES)

    megaXa = nc.dram_tensor("megaXa", [P, (NDC // 2) * B], F8E4,
                            kind="ExternalInput")
    megaXb = nc.dram_tensor("megaXb", [P, (NDC - NDC // 2) * B], F8E4,
                            kind="ExternalInput")
    megaM = nc.dram_tensor("megaM", [P, NDC * MA], F8E4, kind="ExternalInput")
    selT = nc.dram_tensor("selT", [P, NCH * P], F8, kind="ExternalInput")
    maddT = nc.dram_tensor("maddT", [P, NCH * WCAP], F8, kind="ExternalInput")
    cvecT = nc.dram_tensor("cvecT", [P, 4 * NCH], F32, kind="ExternalInput")
    nnT = nc.dram_tensor("nnT", [1, B + MA], BF, kind="ExternalInput")
    identT = nc.dram_tensor("identT", [P, P], F8, kind="ExternalInput")
    out_s = nc.dram_tensor("out_s", [P, 2 * NCH], F32, kind="ExternalOutput")

    with tile.TileContext(nc) as tc:
        with (
            tc.tile_pool(name="cst", bufs=1) as cst,
            tc.tile_pool(name="sm", bufs=4) as sm,
            tc.tile_pool(name="wps", bufs=1, space="PSUM") as wps,
            tc.tile_pool(name="vps", bufs=1, space="PSUM") as vps,
            tc.tile_pool(name="gps", bufs=6, space="PSUM") as gps,
        ):
            # scratch for PE warmup + ACT table preload (no input deps)
            scratch = cst.tile([P, 256], BF)
            nc.vector.memset(scratch[:], 0.25)
            zeros = cst.tile([P, B], BF)
            nc.vector.memset(zeros[:], 0.0)
            zl = cst.tile([P, NCH * B], BF)
            sacc = cst.tile([P, 2 * NCH], F32)

            # input DMAs: contiguous full-tensor DMAs, X split on 2 queues
            mXa = cst.tile([P, (NDC // 2) * B], F8E4)
            nc.sync.dma_start(mXa[:], megaXa[:, :])
            mM = cst.tile([P, NDC * MA], F8E4)
            nc.scalar.dma_start(mM[:], megaM[:, :])
            mXb = cst.tile([P, (NDC - NDC // 2) * B], F8E4)
            nc.scalar.dma_start(mXb[:], megaXb[:, :])
            # tables ride the SWDGE queue (fp8 on the wire, bf16 in SBUF);
            # emission latency naturally defers them behind the X transfers
            nn_sb = cst.tile([1, B + MA], BF)
            nc.gpsimd.dma_start(nn_sb[:], nnT[:, :])
            madd_sb = cst.tile([P, NCH * WCAP], BF)
            nc.gpsimd.dma_start(madd_sb[:], maddT[:, :])
            ident_sb = cst.tile([P, P], BF)
            nc.gpsimd.dma_start(ident_sb[:], identT[:, :])
            sel_sb = cst.tile([P, NCH * P], BF)
            nc.gpsimd.dma_start(sel_sb[:], selT[:, :])
            cvec_sb = cst.tile([P, 4 * NCH], F32)
            nc.gpsimd.dma_start(cvec_sb[:], cvecT[:, :])

            # trigger ACT table load early + warm the PE clock
            tinya = sm.tile([P, 8], BF, tag="tinya")
            nc.scalar.activation(tinya[:], scratch[:, 0:8],
                                 mybir.ActivationFunctionType.Relu)
            warm = wps.tile([P, 256], F32)
            for _ in range(NWARM):
                nc.tensor.matmul(warm[:], lhsT=scratch[:, 0:P],
                                 rhs=scratch[:], start=True, stop=True)

            # V[a,k] = w_a*(2 x_a.x_k - (|x_k|^2 - 512)) in PSUM
            v_sb = cst.tile([P, B], BF)
            nc.gpsimd.memset(v_sb[:], 0.0)
            v_psum = vps.tile([MA, B], F32)
            h2 = NDC // 2
            for dc in range(NDC):
                rhs = (mXa[:, dc * B:(dc + 1) * B] if dc < h2
                       else mXb[:, (dc - h2) * B:(dc - h2 + 1) * B])
                nc.tensor.matmul(v_psum[:],
                                 lhsT=mM[:, dc * MA:(dc + 1) * MA],
                                 rhs=rhs, start=(dc == 0), stop=False)
            nc.tensor.matmul(v_psum[:], lhsT=nn_sb[0:1, B:B + MA],
                             rhs=nn_sb[0:1, 0:B], start=False, stop=True)
            nc.vector.tensor_copy(v_sb[0:MA, :], v_psum[:, :])

            # per-chunk pipeline; sacc col 2c = row-sum, 2c+1 = count
            for c in range(NCH):
                vg = gps.tile([P, B], F32, tag="vg")
                nc.tensor.matmul(vg[:], lhsT=sel_sb[:, c * P:(c + 1) * P],
                                 rhs=v_sb[:], start=True, stop=False)
                nc.tensor.matmul(vg[:, 0:WCAP], lhsT=ident_sb[:],
                                 rhs=madd_sb[:, c * WCAP:(c + 1) * WCAP],
                                 start=False, stop=True)
                zc = zl[:, c * B:(c + 1) * B]
                relu_v, cnt_v = CHUNK_PLAN[c % len(CHUNK_PLAN)]
                if relu_v == "SC":
                    nc.scalar.activation(zc, vg[:],
                                         mybir.ActivationFunctionType.Relu,
                                         bias=cvec_sb[:, c:c + 1], scale=1.0,
                                         accum_out=sacc[:, 2 * c:2 * c + 1])
                else:
                    nc.vector.scalar_tensor_tensor(
                        zc, in0=vg[:], scalar=cvec_sb[:, c:c + 1],
                        in1=zeros[:], op0=mybir.AluOpType.add,
                        op1=mybir.AluOpType.max,
                        accum_out=sacc[:, 2 * c:2 * c + 1])
                cd = sm.tile([P, B], BF, tag="cd")
                if cnt_v == "TS":
                    nc.vector.tensor_scalar(cd[:], zc, float(EPS), None,
                                            op0=mybir.AluOpType.is_gt,
                                            op1=mybir.AluOpType.add,
                                            accum_out=sacc[:, 2 * c + 1:2 * c + 2])
                else:  # SGN: sign(z - EPS) accum = 2*count - B
                    nc.scalar.activation(cd[:], vg[:],
                                         mybir.ActivationFunctionType.Sign,
                                         bias=cvec_sb[:, NCH + c:NCH + c + 1],
                                         scale=1.0,
                                         accum_out=sacc[:, 2 * c + 1:2 * c + 2])
                if c == NCH - 2:
                    nc.sync.dma_start(out_s[:, 0:2 * (NCH - 1)],
                                      sacc[:, 0:2 * (NCH - 1)])
            nc.sync.dma_start(out_s[:, 2 * (NCH - 1):],
                              sacc[:, 2 * (NCH - 1):])

    nc.compile()
    return nc


def run(embeddings, labels, clusters, weights, trace=False):
    tables, NP, MA, WCAP, B, NDC = _host_prep(embeddings, labels, clusters,
                                              weights)
    NCH = NP // P
    nc = _build_program(NCH, MA, WCAP, B, NDC)
    res = run_bass_kernel_spmd(nc, tables, core_ids=list(range(NCORES)),
                               trace=trace)
    S = 0.0
    C = 0.0
    for r in res.results:
        o = np.asarray(r["out_s"], np.float64)
        S += float(o[:, 0::2].sum())
        for c in range(NCH):
            col = float(o[:, 2 * c + 1].sum())
            if CHUNK_PLAN[c % len(CHUNK_PLAN)][1] == "SGN":
                C += (col + P * B) / 2.0
            else:
                C += col
    loss = np.float32(np.float32(S) / np.float32(C + EPS))
    return np.asarray(loss, dtype=np.float32), res


def kernel(embeddings, labels, clusters, weights):
    loss, _ = run(embeddings, labels, clusters, weights)
    return loss
